# revision 12
# baseline (speedup 1.0000x reference)
"""Trainium2 Bass kernel for nn_Encoder_36876589204306 (single-layer
transformer encoder: embed+posenc -> MHA -> add&LN -> FFN -> add&LN).

Sharding: pure data-parallel over batch. B=64 sequences split as 8 per
NeuronCore; every core holds the full weights, no collectives.

v3 design (vs v2 baseline at 340us):
  - Preamble: the old DMA-XBAR transposes + x_nat staging loaded 3.3MB of
    f32 pe and serialized the scalar queue behind the gathers (first
    matmul at 28.8us). Now the xg chunks are transposed on the (idle) PE,
    and the pe/emb residual is injected into the Wo PSUM accumulation via
    two bf16 matmuls per chunk: a SW-scaled identity against the gathered
    emb rows (xg) and a per-chunk permutation matrix against a single
    [100, D] bf16 pe tile (pe repeats every S=100 tokens). This removes
    the pe[N,D] f32 input, the scalar-engine x_nat copies and the DVE
    residual adds entirely.
  - QKV projections in fp8e4 DoubleRow as before (x scale 16, weight
    scale 2048 folded on host, descale 1/32768 on evacuation).
  - Midgame rebalance: the attention phase was DVE-bound (70.8us busy vs
    68 PE). ctxT casts and the Wo z evacuations moved DVE -> scalar.
  - wv is loaded in two ct2-contiguous halves so v_batch(0) starts on
    the first half; wq/wk stream while v runs.
  - Last FFN2 chunk: LN2 fused straight off PSUM (no z2t staging copy).
"""

import numpy as np
import ml_dtypes

import concourse.bass as bass
import concourse.mybir as mybir
import concourse.tile as tile
from concourse import bacc
from concourse.bass import IndirectOffsetOnAxis
from concourse.bass_utils import run_bass_kernel_spmd
from concourse.masks import make_identity

# ---------------- problem dims (hardcoded per contract) ----------------
B, S, D, H, F, V = 64, 100, 1024, 16, 4096, 32000
E = D // H            # 64 head depth
NCORES = 8
BL = B // NCORES      # 8 sequences per core
N = BL * S            # 800 tokens per core
P = 128
DC = D // P           # 8 chunks of d
FC = F // P           # 32 chunks of f
EPS = 1e-6

F32 = mybir.dt.float32
BF = mybir.dt.bfloat16
F8 = mybir.dt.float8e4
AF = mybir.ActivationFunctionType
OP = mybir.AluOpType
DR = mybir.MatmulPerfMode.DoubleRow

N_CH = (N + P - 1) // P                                   # 7 token chunks
CHUNKS = [(c * P, min(P, N - c * P)) for c in range(N_CH)]
VG = 66   # per-head group stride in v_aug (64 v cols + 1 ones col + 1 pad)

SX = 16.0       # fp8 activation scale (folded into emb table + peT on host)
SW = 2048.0     # fp8 weight scale (folded into wq/wk/wv on host)
QSCALE = 1.0 / (SX * SW)


def _bcast(ap, p=P):
    """[n] DRAM AP -> [p, n] partition-broadcast AP."""
    return bass.AP(tensor=ap.tensor, offset=ap.offset, ap=[[0, p]] + list(ap.ap))


def build_nc(flags, dbg=False):
    use_bq = flags["bq"]; use_bk = flags["bk"]; use_bv = flags["bv"]
    use_bo = flags["bo"]; use_b1 = flags["b1"]; use_b2 = flags["b2"]
    use_a1 = flags["a1"]; use_a2 = flags["a2"]

    nc = bacc.Bacc("TRN2", target_bir_lowering=False, debug=False,
                   num_devices=NCORES)

    tokens = nc.dram_tensor("tokens", [P, N_CH], mybir.dt.int32, kind="ExternalInput").ap()
    emb = nc.dram_tensor("emb", [V, D], BF, kind="ExternalInput").ap()
    pe1 = nc.dram_tensor("pe1", [S, D], BF, kind="ExternalInput").ap()
    perm = nc.dram_tensor("perm", [S, N_CH * P], BF, kind="ExternalInput").ap()
    peT = nc.dram_tensor("peT", [P, DC * S], BF, kind="ExternalInput").ap()
    wq = nc.dram_tensor("wq", [P, DC * D], F8, kind="ExternalInput").ap()
    wk = nc.dram_tensor("wk", [P, DC * D], F8, kind="ExternalInput").ap()
    wv = nc.dram_tensor("wv", [P, 2 * DC * 512], F8, kind="ExternalInput").ap()
    wo = nc.dram_tensor("wo", [P, DC * D], F8, kind="ExternalInput").ap()
    w1 = nc.dram_tensor("w1", [P, DC * F], BF, kind="ExternalInput").ap()
    w2 = nc.dram_tensor("w2", [P, FC * D], BF, kind="ExternalInput").ap()
    bq = nc.dram_tensor("bq", [D], F32, kind="ExternalInput").ap() if use_bq else None
    bk = nc.dram_tensor("bk", [D], F32, kind="ExternalInput").ap() if use_bk else None
    bv = nc.dram_tensor("bv", [D], F32, kind="ExternalInput").ap() if use_bv else None
    bo = nc.dram_tensor("bo", [D], F32, kind="ExternalInput").ap() if use_bo else None
    b1 = nc.dram_tensor("b1", [F], F32, kind="ExternalInput").ap() if use_b1 else None
    b2 = nc.dram_tensor("b2", [D], F32, kind="ExternalInput").ap() if use_b2 else None
    g1 = nc.dram_tensor("g1", [D], F32, kind="ExternalInput").ap() if use_a1 else None
    bt1 = nc.dram_tensor("bt1", [D], F32, kind="ExternalInput").ap() if use_a1 else None
    g2 = nc.dram_tensor("g2", [D], F32, kind="ExternalInput").ap() if use_a2 else None
    bt2 = nc.dram_tensor("bt2", [D], F32, kind="ExternalInput").ap() if use_a2 else None
    out = nc.dram_tensor("out", [N, D], F32, kind="ExternalOutput").ap()

    with tile.TileContext(nc) as tc:
        # ---- whole-kernel pools ----
        cpool = tc.alloc_tile_pool(name="const", bufs=1)
        pspool = tc.alloc_tile_pool(name="ps", bufs=8, space="PSUM")
        spool = tc.alloc_tile_pool(name="small", bufs=8)

        epsT = cpool.tile([P, 1], F32, tag="eps")
        nc.vector.memset(epsT, EPS)
        # identity for PE transposes + the FFN2-tail reduction matmuls
        idb = cpool.tile([P, P], BF, tag="idb")
        make_identity(nc, idb)

        # tokens: host pre-arranged [P, N_CH]; contiguous 28B per partition
        tok = cpool.tile([P, N_CH], mybir.dt.int32, tag="tok")
        nc.sync.dma_start(out=tok, in_=tokens)

        # peT (x SX): needed for the xT8 adds right after the gathers
        peT_s = cpool.tile([P, DC, S], BF, tag="peT")
        nc.sync.dma_start(out=peT_s, in_=peT.rearrange("p (c n) -> p c n", c=DC))

        # broadcast tiles for free-axis biases / affines (rarely used)
        def load_bcast(ap_, name, dt=F32, width=D):
            t = cpool.tile([P, width], dt, tag=name)
            nc.sync.dma_start(out=t, in_=_bcast(ap_))
            return t
        bvb = load_bcast(bv, "bvb") if use_bv else None
        bob = load_bcast(bo, "bob") if use_bo else None
        b2b = load_bcast(b2, "b2b") if use_b2 else None
        g1b = load_bcast(g1, "g1b") if use_a1 else None
        bt1b = load_bcast(bt1, "bt1b") if use_a1 else None
        g2b = load_bcast(g2, "g2b") if use_a2 else None
        bt2b = load_bcast(bt2, "bt2b") if use_a2 else None
        bq_s = bk_s = None
        if use_bq:
            bq_s = cpool.tile([P, DC], F32, tag="bq_s")
            nc.sync.dma_start(out=bq_s, in_=bq.rearrange("(c p) -> p c", p=P))
        if use_bk:
            bk_s = cpool.tile([P, DC], F32, tag="bk_s")
            nc.sync.dma_start(out=bk_s, in_=bk.rearrange("(c p) -> p c", p=P))

        # ---- persistent activations ----
        # (alloc order is release order reversed: wpool/x8pool release
        # after the qk second tile; bpool+xgpool after the wo chunks)
        xgpool = tc.alloc_tile_pool(name="xg", bufs=1)
        xg = xgpool.tile([P, N_CH, D], BF, tag="xg")       # gathered emb (x SX)

        bpool = tc.alloc_tile_pool(name="attn_acts", bufs=1)
        qT = bpool.tile([P, DC, N], BF, tag="qT")
        kT = bpool.tile([P, DC, N], BF, tag="kT")
        v_aug = bpool.tile([P, BL, H * VG], BF, tag="v_aug")
        v_r = v_aug.rearrange("p b (h e) -> p b h e", e=VG)
        expT = bpool.tile([P, H, N], BF, tag="expT")
        ctx_nat = bpool.tile([P, BL, D], BF, tag="ctx_nat")
        # rows 100:112 are read (never used) by the padded XBAR transpose;
        # engine base partition must be a multiple of 32 -> clear 96:112
        nc.vector.memset(ctx_nat[96:112], 0.0)
        if dbg:
            nc.vector.memset(v_aug, 0.0)   # pad regions, for debug dumps only
            nc.vector.memset(expT, 0.0)
            nc.vector.memset(ctx_nat, 0.0)

        x8pool = tc.alloc_tile_pool(name="x8p", bufs=1)
        xT8 = x8pool.tile([P, DC, N], F8, tag="xT8")       # fp8 xT (x SX)

        # wv first (earliest matmul consumer), in two ct2-contiguous
        # halves so v_batch(0) starts on half 0
        wpool = tc.alloc_tile_pool(name="wqkv", bufs=3)
        wv_s = wpool.tile([P, 2, DC, 512], F8, tag="wqkv")
        wv_r = wv.rearrange("p (t c n) -> p t c n", t=2, c=DC)
        nc.sync.dma_start(out=wv_s[:, 0], in_=wv_r[:, 0])
        nc.sync.dma_start(out=wv_s[:, 1], in_=wv_r[:, 1])

        # ---- P0: gathers (gpsimd queue) -> PE transposes -> xT8 adds ----
        def gather_issue(c):
            n0, rows = CHUNKS[c]
            nc.gpsimd.indirect_dma_start(
                out=xg[:rows, c, :], out_offset=None, in_=emb,
                in_offset=IndirectOffsetOnAxis(ap=tok[:rows, c:c + 1], axis=0))

        # wq/wk triggered from the gpsimd queue BETWEEN the gather issues:
        # their descriptors then enter the shared DMA rings after the
        # early gathers', so gather c0 (which gates all PE work) isn't
        # starved behind 2MB of weight descriptors in the per-engine FIFOs
        wq_s = wpool.tile([P, DC, D], F8, tag="wqkv")
        wk_s = wpool.tile([P, DC, D], F8, tag="wqkv")
        gather_issue(0)
        gather_issue(1)
        nc.gpsimd.dma_start(out=wq_s, in_=wq.rearrange("p (c n) -> p c n", c=DC))
        gather_issue(2)
        gather_issue(3)
        nc.gpsimd.dma_start(out=wk_s, in_=wk.rearrange("p (c n) -> p c n", c=DC))
        gather_issue(4)
        gather_issue(5)
        gather_issue(6)

        # SW-scaled identity for the Wo-phase emb-residual matmuls
        # (built on gpsimd behind the gather issues; needed only ~90us in)
        idsw = cpool.tile([P, P], BF, tag="idsw")
        nc.vector.memset(idsw, 0.0)
        nc.gpsimd.affine_select(
            out=idsw, in_=idsw, compare_op=OP.not_equal, fill=SW,
            base=0, pattern=[[-1, P]], channel_multiplier=1)

        # pe1/perm for the Wo-phase pe-residual matmuls; issued after
        # wq/wk so they don't contend with the preamble critical path
        pe1_s = cpool.tile([P, D], BF, tag="pe1")
        nc.sync.dma_start(out=pe1_s[:S], in_=pe1)
        perm_s = cpool.tile([P, N_CH, P], BF, tag="perm")
        nc.sync.dma_start(out=perm_s[:S],
                          in_=perm.rearrange("p (c r) -> p c r", c=N_CH))

        def xpose_chunk(c):
            """PE-transpose xg chunk c, then xT8 = fp8(x*SX + pe*SX)."""
            n0, rows = CHUNKS[c]
            pst = pspool.tile([P, DC, P], BF, tag="ps")
            for d in range(DC):
                nc.tensor.transpose(out=pst[:, d, 0:rows],
                                    in_=xg[:rows, c, d * P:(d + 1) * P],
                                    identity=idb[:rows, :rows])
            segs, off = [], 0
            while off < rows:
                s = (n0 + off) % S
                ln = min(rows - off, S - s)
                segs.append((off, s, ln))
                off += ln
            for (off, s, ln) in segs:
                nc.vector.tensor_add(
                    out=xT8[:, :, n0 + off:n0 + off + ln],
                    in0=pst[:, :, off:off + ln],
                    in1=peT_s[:, :, s:s + ln])

        # ---- QKV in fp8 DoubleRow ----
        def qk8(w_s, dst, b_s, on_scalar, t0, tw):
            for ct in range(DC):
                ps = pspool.tile([P, 512], F32, tag="ps")
                for k2 in range(DC // 2):
                    nc.tensor.matmul(ps[:, :tw],
                                     lhsT=w_s[:, 2 * k2:2 * k2 + 2,
                                              ct * P:(ct + 1) * P],
                                     rhs=xT8[:, 2 * k2:2 * k2 + 2, t0:t0 + tw],
                                     start=(k2 == 0), stop=(k2 == DC // 2 - 1),
                                     perf_mode=DR)
                if b_s is not None:
                    nc.scalar.activation(out=dst[:, ct, t0:t0 + tw],
                                         in_=ps[:, :tw], func=AF.Copy,
                                         bias=b_s[:, ct:ct + 1], scale=QSCALE)
                elif on_scalar:
                    nc.scalar.activation(out=dst[:, ct, t0:t0 + tw],
                                         in_=ps[:, :tw], func=AF.Copy,
                                         scale=QSCALE)
                else:
                    nc.vector.tensor_scalar_mul(out=dst[:, ct, t0:t0 + tw],
                                                in0=ps[:, :tw], scalar1=QSCALE)

        def v_batch(b):
            for ct2 in range(2):
                ps = pspool.tile([P, 512], F32, tag="ps")
                for k2 in range(DC // 2):
                    nc.tensor.matmul(ps[:S, :],
                                     lhsT=xT8[:, 2 * k2:2 * k2 + 2,
                                              b * S:(b + 1) * S],
                                     rhs=wv_s[:, ct2, 2 * k2:2 * k2 + 2, :],
                                     start=(k2 == 0), stop=(k2 == DC // 2 - 1),
                                     perf_mode=DR)
                dstv = v_r[:S, b, ct2 * 8:(ct2 + 1) * 8, 0:64]
                psv = ps[:S, :].rearrange("p (h e) -> p h e", e=64)
                if use_bv:
                    sc = spool.tile([P, 512], F32, tag="vsc", bufs=2)
                    nc.vector.tensor_scalar_mul(out=sc[:S], in0=ps[:S, :],
                                                scalar1=QSCALE)
                    nc.vector.tensor_add(
                        out=dstv, in0=sc[:S].rearrange("p (h e) -> p h e", e=64),
                        in1=bvb[:S, ct2 * 512:(ct2 + 1) * 512]
                            .rearrange("p (h e) -> p h e", e=64))
                else:
                    nc.vector.tensor_scalar_mul(out=dstv, in0=psv,
                                                scalar1=QSCALE)
            # 1/SX in the sums column: the reciprocal then yields SX/sum,
            # so ctx_nat carries a factor SX, sized for the fp8 ctxT cast
            nc.vector.memset(v_r[:S, b, :, 64:65], 1.0 / SX)

        def scores_group(hq, bq4):
            # heads 4hq..4hq+3 as two even/odd pairs; even head sits at
            # partition 0, odd at 64 -> distinct PE row groups, MMs overlap
            for pr in range(2):
                h0, h1 = hq * 4 + 2 * pr, hq * 4 + 2 * pr + 1
                pch = h0 // 2
                psA = pspool.tile([P, 4, S], F32, tag="ps")
                psB = pspool.tile([P, 4, S], F32, tag="ps")
                for j in range(4):
                    b = bq4 * 4 + j
                    sl = slice(b * S, (b + 1) * S)
                    nc.tensor.matmul(psA[:S, j, :], lhsT=kT[0:64, pch, sl],
                                     rhs=qT[0:64, pch, sl],
                                     start=True, stop=True)
                    nc.tensor.matmul(psB[:S, j, :], lhsT=kT[64:128, pch, sl],
                                     rhs=qT[64:128, pch, sl],
                                     start=True, stop=True)
                for h, psx in ((h0, psA), (h1, psB)):
                    nc.scalar.activation(
                        out=expT[:S, h, bq4 * 4 * S:(bq4 * 4 + 4) * S]
                            .rearrange("p (j s) -> p j s", s=S),
                        in_=psx[:S], func=AF.Exp, scale=float(1.0 / np.sqrt(E)))

        def ctx_b(b):
            # all 16 heads of batch b, then its ctxT transpose on the
            # (midgame-idle) DMA XBAR via the sync queue
            for hq in range(4):
                ps = pspool.tile([P, 4, VG], F32, tag="ps")
                for j in range(4):
                    h = hq * 4 + j
                    nc.tensor.matmul(ps[:S, j, 0:65],
                                     lhsT=expT[:S, h, b * S:(b + 1) * S],
                                     rhs=v_r[:S, b, h, 0:65],
                                     start=True, stop=True)
                rc = spool.tile([P, 4], F32, tag="rc")
                nc.vector.reciprocal(out=rc[:S], in_=ps[:S, :, 64])
                # one op for all 4 heads: rc broadcast along e via a
                # zero-stride free dim
                rcs = rc[:S, 0:4]
                rcb = bass.AP(tensor=rcs.tensor, offset=rcs.offset,
                              ap=list(rcs.ap) + [[0, 64]])
                nc.vector.tensor_mul(
                    out=ctx_nat[:S, b, hq * 256:(hq + 1) * 256]
                        .rearrange("p (j e) -> p j e", e=64),
                    in0=ps[:S, :, 0:64], in1=rcb)
            # XBAR needs p_dim % 16 == 0: read 112 rows (12 garbage rows
            # land in stage cols 100:112, never read); fp8 cast on scalar
            st = ctpool.tile([P, DC, 112], BF, tag="cst")
            nc.sync.dma_start(out=st, in_=ctx_nat[0:112, b, :], transpose=True)
            nc.scalar.activation(out=ctxT[:, :, b * S:(b + 1) * S],
                                 in_=st[:, :, 0:100], func=AF.Copy)

        # PE order: transposes c0-c3 interleave with v batches (each v
        # batch needs only the chunks covering its tokens); the qk tiles
        # then cover chunks 0-3 while c4-c6 gathers land
        xpose_chunk(0)
        v_batch(0)
        xpose_chunk(1)
        v_batch(1)
        xpose_chunk(2)
        v_batch(2)
        xpose_chunk(3)
        v_batch(3)
        qk8(wq_s, qT, bq_s, True, 0, 512)
        qk8(wk_s, kT, bk_s, False, 0, 512)
        xpose_chunk(4)
        xpose_chunk(5)
        xpose_chunk(6)

        # mid-phase allocations (right stack, LIFO: ctpool released first,
        # then wopool, mpool; x1fpool survives through FFN2)
        x1fpool = tc.alloc_tile_pool(name="x1f", bufs=1, side="right")
        x1b = x1fpool.tile([P, N_CH, D], BF, tag="x1b")
        x1T = x1fpool.tile([P, DC, N], BF, tag="x1T")
        mpool = tc.alloc_tile_pool(name="mid", bufs=1, side="right")
        ctxT = mpool.tile([P, DC, N], F8, tag="ctxT")
        # z in bf16: halves the LN1 DVE read traffic; the bf16 rounding
        # is well inside the error budget
        z = mpool.tile([P, N_CH, D], BF, tag="z")
        wopool = tc.alloc_tile_pool(name="wop", bufs=1, side="right")
        wo_s = wopool.tile([P, DC, D], F8, tag="wo")
        nc.sync.dma_start(out=wo_s, in_=wo.rearrange("p (c n) -> p c n", c=DC))
        ctpool = tc.alloc_tile_pool(name="ctstg", bufs=2, side="right")

        for hq in range(4):
            scores_group(hq, 0)
        # ctx batches 0-3 and v4 depend only on bq0 scores — they fill
        # the PE while the second qk tile waits on chunks 4-6
        ctx_b(0)
        ctx_b(1)
        ctx_b(2)
        ctx_b(3)
        v_batch(4)
        v_batch(5)
        v_batch(6)
        qk8(wq_s, qT, bq_s, True, 512, N - 512)
        qk8(wk_s, kT, bk_s, False, 512, N - 512)

        def wo_chunk(c):
            n0, rows = CHUNKS[c]
            for ct2 in range(2):
                ps = pspool.tile([P, 512], F32, tag="ps")
                for k2 in range(DC // 2):
                    nc.tensor.matmul(ps[:rows],
                                     lhsT=ctxT[:, 2 * k2:2 * k2 + 2,
                                               n0:n0 + rows],
                                     rhs=wo_s[:, 2 * k2:2 * k2 + 2,
                                              ct2 * 512:(ct2 + 1) * 512],
                                     start=(k2 == 0), stop=False,
                                     perf_mode=DR)
                # residual: += SX*SW*emb via the SW-scaled identity over
                # xg (which carries SX), then += SX*SW*pe via the chunk's
                # row-permutation against the single [100, D] pe tile
                nc.tensor.matmul(ps[:rows], lhsT=idsw[:rows, :rows],
                                 rhs=xg[:rows, c, ct2 * 512:(ct2 + 1) * 512],
                                 start=False, stop=False)
                nc.tensor.matmul(ps[:rows], lhsT=perm_s[:S, c, 0:rows],
                                 rhs=pe1_s[:S, ct2 * 512:(ct2 + 1) * 512],
                                 start=False, stop=True)
                nc.scalar.activation(out=z[:rows, c, ct2 * 512:(ct2 + 1) * 512],
                                     in_=ps[:rows], func=AF.Copy, scale=QSCALE)
            if use_bo:
                nc.vector.tensor_add(out=z[:rows, c, :], in0=z[:rows, c, :],
                                     in1=bob[:rows])

        def resid_mm(ps, rows, resid):
            # += residual via identity matmul (diag-hit contraction over
            # the token partitions); closes the accumulation group
            nc.tensor.matmul(ps[:rows], lhsT=idb[:rows, :rows], rhs=resid,
                             start=False, stop=True)

        # LN1 stats helpers (needed early: the stats thread into the wo
        # chain below; the normalize half lives after the x1 pools)
        mvs = []
        def ln1_stats(c):
            rows = CHUNKS[c][1]
            st = spool.tile([P, 2, 6], F32, tag="st")
            mv = spool.tile([P, 2], F32, tag="mv")
            src = z[:rows, c, :]
            nc.vector.bn_stats(out=st[:rows, 0, :], in_=src[:, 0:512])
            nc.vector.bn_stats(out=st[:rows, 1, :], in_=src[:, 512:1024])
            nc.vector.bn_aggr(out=mv[:rows], in_=st[:rows])
            mvs.append(mv)

        def ln1_sqrt(c):
            rows = CHUNKS[c][1]
            nc.scalar.activation(out=mvs[c][:rows, 1:2], in_=mvs[c][:rows, 1:2],
                                 func=AF.Sqrt, bias=epsT[:rows], scale=1.0)

        def ln1_norm(c):
            n0, rows = CHUNKS[c]
            mv = mvs[c]
            nc.vector.reciprocal(out=mv[:rows, 1:2], in_=mv[:rows, 1:2])
            nc.vector.tensor_scalar(out=x1b[:rows, c, :], in0=z[:rows, c, :],
                                    scalar1=mv[:rows, 0:1],
                                    scalar2=mv[:rows, 1:2],
                                    op0=OP.subtract, op1=OP.mult)
            if use_a1:
                nc.vector.tensor_mul(out=x1b[:rows, c, :],
                                     in0=x1b[:rows, c, :], in1=g1b[:rows])
                nc.vector.tensor_add(out=x1b[:rows, c, :],
                                     in0=x1b[:rows, c, :], in1=bt1b[:rows])
            # x1T on the DMA XBAR: no PE, no PSUM, no DVE copy
            nc.sync.dma_start(out=x1T[:, :, n0:n0 + rows],
                              in_=x1b[:rows, c, :], transpose=True)

        # interleave: fp8 v batches and ACT-paced scores groups spread
        # against the vector-paced ctx evacuations; wo chunks start as soon
        # as their two ctxT batches exist; the LN1 chain (stats -> sqrt ->
        # norm -> XBAR transpose) threads in behind them so x1T chunks are
        # ready the moment FFN1 starts
        scores_group(0, 1)
        wo_chunk(0)
        scores_group(1, 1)
        v_batch(7)
        scores_group(2, 1)
        wo_chunk(1)
        scores_group(3, 1)
        ln1_stats(0)
        ln1_sqrt(0)
        ctx_b(4)
        wo_chunk(2)
        ln1_stats(1)
        ln1_sqrt(1)
        ln1_norm(0)
        ctx_b(5)
        wo_chunk(3)
        ln1_stats(2)
        ln1_sqrt(2)
        ln1_norm(1)
        ctx_b(6)
        wo_chunk(4)
        ln1_stats(3)
        ln1_sqrt(3)
        ln1_norm(2)
        ctx_b(7)
        wo_chunk(5)
        ln1_norm(3)
        wo_chunk(6)
        ln1_stats(4)
        ln1_sqrt(4)
        ln1_norm(4)
        ln1_stats(5)
        ln1_sqrt(5)
        ln1_norm(5)
        ln1_stats(6)
        ln1_sqrt(6)
        ln1_norm(6)
        if dbg:
            d_xT8 = nc.dram_tensor("d_xT8", [P, DC * N], F8, kind="ExternalOutput").ap()
            d_qT = nc.dram_tensor("d_qT", [P, DC * N], BF, kind="ExternalOutput").ap()
            d_kT = nc.dram_tensor("d_kT", [P, DC * N], BF, kind="ExternalOutput").ap()
            d_v = nc.dram_tensor("d_v", [P, BL * H * VG], BF, kind="ExternalOutput").ap()
            d_exp = nc.dram_tensor("d_exp", [P, H * N], BF, kind="ExternalOutput").ap()
            d_ctx = nc.dram_tensor("d_ctx", [P, BL * D], BF, kind="ExternalOutput").ap()
            d_ctxT = nc.dram_tensor("d_ctxT", [P, DC * N], F8, kind="ExternalOutput").ap()
            d_z = nc.dram_tensor("d_z", [P, N_CH * D], BF, kind="ExternalOutput").ap()
            nc.sync.dma_start(out=d_xT8, in_=xT8.rearrange("p c n -> p (c n)"))
            nc.sync.dma_start(out=d_qT, in_=qT.rearrange("p c n -> p (c n)"))
            nc.sync.dma_start(out=d_kT, in_=kT.rearrange("p c n -> p (c n)"))
            nc.sync.dma_start(out=d_v, in_=v_aug.rearrange("p b h -> p (b h)"))
            nc.sync.dma_start(out=d_exp, in_=expT.rearrange("p h n -> p (h n)"))
            nc.sync.dma_start(out=d_ctx, in_=ctx_nat.rearrange("p b d -> p (b d)"))
            nc.sync.dma_start(out=d_ctxT, in_=ctxT.rearrange("p c n -> p (c n)"))
            nc.sync.dma_start(out=d_z, in_=z.rearrange("p c d -> p (c d)"))
        ctpool.release()
        wopool.release()
        mpool.release()
        wpool.release()
        x8pool.release()
        bpool.release()
        xgpool.release()

        # ---- FFN1: h1T = relu(W1.T @ x1T + b1)  (bf16, T layout) ----
        w2pool = tc.alloc_tile_pool(name="w2p", bufs=1)
        w2_s = w2pool.tile([P, FC, D], BF, tag="w2s")
        hpool = tc.alloc_tile_pool(name="h1", bufs=1, side="right")
        h1T = hpool.tile([P, FC, N], BF, tag="h1T")
        b1_s = None
        if use_b1:
            b1_s = cpool.tile([P, FC], F32, tag="b1_s")
            nc.sync.dma_start(out=b1_s, in_=b1.rearrange("(c p) -> p c", p=P))
        w1_r = w1.rearrange("p (c f) -> p c f", c=DC)
        N_TILES = [(0, 512), (512, N - 512)]
        with tc.tile_pool(name="w1s", bufs=4) as w1pool:
            def f1_mm(w1t, fg, fc4, t0, tw):
                fabs = fg * 4 + fc4
                ps = pspool.tile([P, 512], F32, tag="ps")
                for kc in range(DC):
                    nc.tensor.matmul(ps[:, :tw],
                                     lhsT=w1t[:, kc, fc4 * P:(fc4 + 1) * P],
                                     rhs=x1T[:, kc, t0:t0 + tw],
                                     start=(kc == 0), stop=(kc == DC - 1))
                if use_b1:
                    nc.scalar.activation(out=h1T[:, fabs, t0:t0 + tw],
                                         in_=ps[:, :tw], func=AF.Relu,
                                         bias=b1_s[:, fabs:fabs + 1], scale=1.0)
                else:
                    nc.scalar.activation(out=h1T[:, fabs, t0:t0 + tw],
                                         in_=ps[:, :tw], func=AF.Relu)

            # two passes: all f-groups on n-tile 0 (tokens 0-512) first, so
            # the trailing LN1 chunks 4-6 and their transposes hide under
            # ~50us of pass-1 matmuls; w1 is streamed twice (DMA is cheap)
            w2_r = w2.rearrange("p (c n) -> p c n", c=FC)
            for pi, (t0, tw) in enumerate(N_TILES):
                for fg in range(8):
                    # w2 halves ride in pass-1's DMA slack (pass-2 has none)
                    if pi == 0 and fg == 3:
                        nc.sync.dma_start(out=w2_s[:, 0:16, :],
                                          in_=w2_r[:, 0:16, :])
                    elif pi == 0 and fg == 6:
                        nc.sync.dma_start(out=w2_s[:, 16:32, :],
                                          in_=w2_r[:, 16:32, :])
                    w1t = w1pool.tile([P, DC, 512], BF, tag="w1t")
                    nc.sync.dma_start(out=w1t,
                                      in_=w1_r[:, :, fg * 512:(fg + 1) * 512])
                    if pi == 0 and fg == 0:
                        # first f-group in two 256-token sub-tiles: the
                        # first needs only x1T chunks 0-1, so FFN1 starts
                        # a couple of LN1-norms earlier
                        for fc4 in range(4):
                            f1_mm(w1t, fg, fc4, 0, 256)
                        for fc4 in range(4):
                            f1_mm(w1t, fg, fc4, 256, 256)
                    else:
                        for fc4 in range(4):
                            f1_mm(w1t, fg, fc4, t0, tw)

        # ---- FFN2 + residual + LN2 -> out ----
        # tail chunk (32 rows) first: its packed-reduction latency hides
        # under the full chunks instead of extending the kernel tail.
        # The final chunk's LN2 runs straight off PSUM (no z2t copy).
        opool = tc.alloc_tile_pool(name="ostage", bufs=3)
        order = [N_CH - 1] + list(range(N_CH - 1))
        for oi, c in enumerate(order):
            last = oi == len(order) - 1
            n0, rows = CHUNKS[c]
            z2t = None if last else opool.tile([P, D], F32, tag="z2")
            st = spool.tile([P, 2, 6], F32, tag="st")
            pss = []
            for ct2 in range(2):
                ps = pspool.tile([P, 512], F32, tag="ps")
                if rows == P:
                    for kc in range(FC):
                        nc.tensor.matmul(ps[:rows],
                                         lhsT=h1T[:, kc, n0:n0 + rows],
                                         rhs=w2_s[:, kc,
                                                  ct2 * 512:(ct2 + 1) * 512],
                                         start=(kc == 0), stop=False)
                    resid_mm(ps, rows,
                             x1b[:rows, c, ct2 * 512:(ct2 + 1) * 512])
                    if last:
                        nc.vector.bn_stats(out=st[:rows, ct2, :], in_=ps[:rows])
                        pss.append(ps)
                    else:
                        nc.vector.tensor_copy(
                            out=z2t[:rows, ct2 * 512:(ct2 + 1) * 512],
                            in_=ps[:rows])
                        nc.vector.bn_stats(
                            out=st[:rows, ct2, :],
                            in_=z2t[:rows, ct2 * 512:(ct2 + 1) * 512])
                else:
                    # 32-row tail: 4 col-groups accumulate 8-kc partial sums
                    # concurrently (kk outer interleaves the chains); the
                    # partition groups are then summed with 4 accumulating
                    # identity-slice matmuls (walrus forbids cross-partition
                    # DVE operands, the PE reduction sidesteps that)
                    for kk in range(8):
                        for g in range(4):
                            kc = g * 8 + kk
                            nc.tensor.matmul(
                                ps[g * 32:g * 32 + 32, :],
                                lhsT=h1T[:, kc, n0:n0 + rows],
                                rhs=w2_s[:, kc, ct2 * 512:(ct2 + 1) * 512],
                                start=(kk == 0), stop=(kk == 7),
                                tile_position=(0, g * 32),
                                skip_group_check=True)
                    tsb = spool.tile([P, 512], BF, tag="acc", bufs=2)
                    nc.vector.tensor_copy(out=tsb, in_=ps)
                    ps2 = pspool.tile([P, 512], F32, tag="ps")
                    for g in range(4):
                        nc.tensor.matmul(ps2[0:32, :],
                                         lhsT=idb[:, g * 32:g * 32 + 32],
                                         rhs=tsb,
                                         start=(g == 0), stop=False)
                    resid_mm(ps2, rows,
                             x1b[:rows, c, ct2 * 512:(ct2 + 1) * 512])
                    nc.vector.tensor_copy(
                        out=z2t[:rows, ct2 * 512:(ct2 + 1) * 512],
                        in_=ps2[:rows])
                    nc.vector.bn_stats(
                        out=st[:rows, ct2, :],
                        in_=z2t[:rows, ct2 * 512:(ct2 + 1) * 512])
            if use_b2:
                nc.vector.tensor_add(out=z2t[:rows], in0=z2t[:rows], in1=b2b[:rows])
                nc.vector.bn_stats(out=st[:rows, 0, :], in_=z2t[:rows, 0:512])
                nc.vector.bn_stats(out=st[:rows, 1, :], in_=z2t[:rows, 512:1024])
            ot = opool.tile([P, D], F32, tag="ot")
            mv = spool.tile([P, 2], F32, tag="mv")
            nc.vector.bn_aggr(out=mv[:rows], in_=st[:rows])
            nc.scalar.activation(out=mv[:rows, 1:2], in_=mv[:rows, 1:2],
                                 func=AF.Sqrt, bias=epsT[:rows], scale=1.0)
            nc.vector.reciprocal(out=mv[:rows, 1:2], in_=mv[:rows, 1:2])
            # normalize + store per half so the final DMA overlaps the
            # second half's normalize
            for h2 in range(2):
                sl = slice(h2 * 512, (h2 + 1) * 512)
                src = pss[h2][:rows] if last else z2t[:rows, sl]
                nc.vector.tensor_scalar(out=ot[:rows, sl], in0=src,
                                        scalar1=mv[:rows, 0:1],
                                        scalar2=mv[:rows, 1:2],
                                        op0=OP.subtract, op1=OP.mult)
                if use_a2:
                    nc.vector.tensor_mul(out=ot[:rows, sl], in0=ot[:rows, sl],
                                         in1=g2b[:rows, sl])
                    nc.vector.tensor_add(out=ot[:rows, sl], in0=ot[:rows, sl],
                                         in1=bt2b[:rows, sl])
                nc.sync.dma_start(out=out[n0:n0 + rows, sl],
                                  in_=ot[:rows, sl])

        opool.release()
        w2pool.release()
        hpool.release()
        x1fpool.release()
        spool.release()
        pspool.release()
        cpool.release()

    nc.compile()
    return nc


# ---------------- host side ----------------

def _positional_encoding(seq_len, dim):
    pos = np.arange(seq_len).reshape(seq_len, 1).astype(np.float64)
    i = np.arange(dim)
    div_term = np.power(10000.0, 2 * (i // 2) / dim)
    pe = np.zeros((seq_len, dim))
    pe[:, 0::2] = np.sin(pos / div_term[0::2])
    pe[:, 1::2] = np.cos(pos / div_term[1::2])
    return pe.astype(np.float32)


_NC_CACHE = {}


def _get_nc(flags):
    key = tuple(sorted(flags.items()))
    if key not in _NC_CACHE:
        _NC_CACHE[key] = build_nc(flags)
    return _NC_CACHE[key]


def make_in_maps(tokens, emb_table, Wq, bq, Wk, bk, Wv, bv, Wo, bo,
                 W1, b1, W2, b2, gamma1, beta1, gamma2, beta2):
    bf16 = ml_dtypes.bfloat16
    fp8 = ml_dtypes.float8_e4m3
    f32 = np.float32

    def arrange(w, nchunk):  # [rows, n] -> [P, nchunk*n] in SBUF layout
        rows, n = w.shape
        return np.ascontiguousarray(
            w.reshape(nchunk, P, n).swapaxes(0, 1).reshape(P, nchunk * n))

    def merge_hw(w):  # [H, D, E] -> [D, H*E]
        return np.transpose(np.asarray(w, f32), (1, 0, 2)).reshape(D, D)

    def to_fp8(w):  # scale, clip below e4m3 max-finite, quantize
        return np.clip(w * SW, -240.0, 240.0).astype(fp8)

    flags = {
        "bq": bool(np.any(np.asarray(bq))), "bk": bool(np.any(np.asarray(bk))),
        "bv": bool(np.any(np.asarray(bv))), "bo": bool(np.any(np.asarray(bo))),
        "b1": bool(np.any(np.asarray(b1))), "b2": bool(np.any(np.asarray(b2))),
        "a1": not (np.all(np.asarray(gamma1) == 1.0) and not np.any(np.asarray(beta1))),
        "a2": not (np.all(np.asarray(gamma2) == 1.0) and not np.any(np.asarray(beta2))),
    }

    pe1 = _positional_encoding(S, D)
    # wv in two ct2-contiguous halves (the kernel loads them separately)
    wv_a = arrange(to_fp8(merge_hw(Wv)), DC)                    # [P, DC*D]
    wv_a = wv_a.reshape(P, DC, 2, 512).transpose(0, 2, 1, 3)    # [P, 2, DC, 512]
    # chunk-row permutation matrices: perm[s, c*128+r] = 1 iff the pe row
    # for token c*128+r is s (pe repeats every S tokens)
    perm = np.zeros((S, N_CH * P), f32)
    for c in range(N_CH):
        n0, rows = c * P, min(P, N - c * P)
        r = np.arange(rows)
        perm[(n0 + r) % S, c * P + r] = 1.0
    common = {
        "emb": (np.asarray(emb_table, f32) * SX).astype(bf16),
        "pe1": (pe1 * np.float32(SX * SW)).astype(bf16),
        "perm": perm.astype(bf16),
        "peT": arrange(np.ascontiguousarray(pe1.T * SX).astype(bf16), DC),
        "wq": arrange(to_fp8(merge_hw(Wq)), DC),
        "wk": arrange(to_fp8(merge_hw(Wk)), DC),
        "wv": np.ascontiguousarray(wv_a.reshape(P, 2 * DC * 512)),
        "wo": arrange(to_fp8(np.asarray(Wo, f32)), DC),
        "w1": arrange(np.asarray(W1, f32).astype(bf16), DC),
        "w2": arrange(np.asarray(W2, f32).astype(bf16), FC),
    }
    if flags["bq"]: common["bq"] = np.asarray(bq, f32).reshape(D)
    if flags["bk"]: common["bk"] = np.asarray(bk, f32).reshape(D)
    if flags["bv"]: common["bv"] = np.asarray(bv, f32).reshape(D)
    if flags["bo"]: common["bo"] = np.asarray(bo, f32).reshape(D)
    if flags["b1"]: common["b1"] = np.asarray(b1, f32).reshape(F)
    if flags["b2"]: common["b2"] = np.asarray(b2, f32).reshape(D)
    if flags["a1"]:
        common["g1"] = np.asarray(gamma1, f32).reshape(D)
        common["bt1"] = np.asarray(beta1, f32).reshape(D)
    if flags["a2"]:
        common["g2"] = np.asarray(gamma2, f32).reshape(D)
        common["bt2"] = np.asarray(beta2, f32).reshape(D)

    tokens = np.asarray(tokens, np.int32)
    in_maps = []
    for i in range(NCORES):
        flat = tokens[i * BL:(i + 1) * BL].reshape(N)
        padded = np.zeros(N_CH * P, np.int32)
        padded[:N] = flat
        m = dict(common)
        # tok[p, c] = token index c*128+p
        m["tokens"] = np.ascontiguousarray(padded.reshape(N_CH, P).T)
        in_maps.append(m)
    return flags, in_maps


def kernel(**inputs):
    flags, in_maps = make_in_maps(**inputs)
    nc = _get_nc(flags)
    res = run_bass_kernel_spmd(nc, in_maps, list(range(NCORES)))
    outs = [np.asarray(res.results[i]["out"], np.float32).reshape(BL, S, D)
            for i in range(NCORES)]
    return np.concatenate(outs, axis=0)


# revision 20
# speedup vs baseline: 1.0268x; 1.0268x over previous
"""Trainium2 Bass kernel for nn_Encoder_36876589204306 (single-layer
transformer encoder: embed+posenc -> MHA -> add&LN -> FFN -> add&LN).

Sharding: pure data-parallel over batch. B=64 sequences split as 8 per
NeuronCore; every core holds the full weights, no collectives.

v3 design (vs v2 baseline at 340us):
  - Preamble: the old DMA-XBAR transposes + x_nat staging loaded 3.3MB of
    f32 pe and serialized the scalar queue behind the gathers (first
    matmul at 28.8us). Now the xg chunks are transposed on the (idle) PE,
    and the pe/emb residual is injected into the Wo PSUM accumulation via
    two bf16 matmuls per chunk: a SW-scaled identity against the gathered
    emb rows (xg) and a per-chunk permutation matrix against a single
    [100, D] bf16 pe tile (pe repeats every S=100 tokens). This removes
    the pe[N,D] f32 input, the scalar-engine x_nat copies and the DVE
    residual adds entirely.
  - QKV projections in fp8e4 DoubleRow as before (x scale 16, weight
    scale 2048 folded on host, descale 1/32768 on evacuation).
  - Midgame rebalance: the attention phase was DVE-bound (70.8us busy vs
    68 PE). ctxT casts and the Wo z evacuations moved DVE -> scalar.
  - wv is loaded in two ct2-contiguous halves so v_batch(0) starts on
    the first half; wq/wk stream while v runs.
  - Last FFN2 chunk: LN2 fused straight off PSUM (no z2t staging copy).
"""

import numpy as np
import ml_dtypes

import concourse.bass as bass
import concourse.mybir as mybir
import concourse.tile as tile
from concourse import bacc
from concourse.bass import IndirectOffsetOnAxis
from concourse.bass_utils import run_bass_kernel_spmd
from concourse.masks import make_identity

# ---------------- problem dims (hardcoded per contract) ----------------
B, S, D, H, F, V = 64, 100, 1024, 16, 4096, 32000
E = D // H            # 64 head depth
NCORES = 8
BL = B // NCORES      # 8 sequences per core
N = BL * S            # 800 tokens per core
P = 128
DC = D // P           # 8 chunks of d
FC = F // P           # 32 chunks of f
EPS = 1e-6

F32 = mybir.dt.float32
BF = mybir.dt.bfloat16
F8 = mybir.dt.float8e4
AF = mybir.ActivationFunctionType
OP = mybir.AluOpType
DR = mybir.MatmulPerfMode.DoubleRow

N_CH = (N + P - 1) // P                                   # 7 token chunks
CHUNKS = [(c * P, min(P, N - c * P)) for c in range(N_CH)]
VG = 66   # per-head group stride in v_aug (64 v cols + 1 ones col + 1 pad)

SX = 16.0       # fp8 activation scale (folded into emb table + peT on host)
SW = 2048.0     # fp8 weight scale (folded into wq/wk/wv on host)
QSCALE = 1.0 / (SX * SW)


def _bcast(ap, p=P):
    """[n] DRAM AP -> [p, n] partition-broadcast AP."""
    return bass.AP(tensor=ap.tensor, offset=ap.offset, ap=[[0, p]] + list(ap.ap))


def build_nc(flags, dbg=False):
    use_bq = flags["bq"]; use_bk = flags["bk"]; use_bv = flags["bv"]
    use_bo = flags["bo"]; use_b1 = flags["b1"]; use_b2 = flags["b2"]
    use_a1 = flags["a1"]; use_a2 = flags["a2"]

    nc = bacc.Bacc("TRN2", target_bir_lowering=False, debug=False,
                   num_devices=NCORES)

    tokens = nc.dram_tensor("tokens", [P, N_CH], mybir.dt.int32, kind="ExternalInput").ap()
    emb = nc.dram_tensor("emb", [V, D], BF, kind="ExternalInput").ap()
    pe1 = nc.dram_tensor("pe1", [S, D], BF, kind="ExternalInput").ap()
    perm = nc.dram_tensor("perm", [S, N_CH * P], BF, kind="ExternalInput").ap()
    peT = nc.dram_tensor("peT", [P, DC * S], BF, kind="ExternalInput").ap()
    wq = nc.dram_tensor("wq", [P, DC * D], F8, kind="ExternalInput").ap()
    wk = nc.dram_tensor("wk", [P, DC * D], F8, kind="ExternalInput").ap()
    wv = nc.dram_tensor("wv", [P, 2 * DC * 512], F8, kind="ExternalInput").ap()
    wo = nc.dram_tensor("wo", [P, DC * D], F8, kind="ExternalInput").ap()
    w1 = nc.dram_tensor("w1", [P, DC * F], BF, kind="ExternalInput").ap()
    w2 = nc.dram_tensor("w2", [P, FC * D], BF, kind="ExternalInput").ap()
    bq = nc.dram_tensor("bq", [D], F32, kind="ExternalInput").ap() if use_bq else None
    bk = nc.dram_tensor("bk", [D], F32, kind="ExternalInput").ap() if use_bk else None
    bv = nc.dram_tensor("bv", [D], F32, kind="ExternalInput").ap() if use_bv else None
    bo = nc.dram_tensor("bo", [D], F32, kind="ExternalInput").ap() if use_bo else None
    b1 = nc.dram_tensor("b1", [F], F32, kind="ExternalInput").ap() if use_b1 else None
    b2 = nc.dram_tensor("b2", [D], F32, kind="ExternalInput").ap() if use_b2 else None
    g1 = nc.dram_tensor("g1", [D], F32, kind="ExternalInput").ap() if use_a1 else None
    bt1 = nc.dram_tensor("bt1", [D], F32, kind="ExternalInput").ap() if use_a1 else None
    g2 = nc.dram_tensor("g2", [D], F32, kind="ExternalInput").ap() if use_a2 else None
    bt2 = nc.dram_tensor("bt2", [D], F32, kind="ExternalInput").ap() if use_a2 else None
    out = nc.dram_tensor("out", [N, D], F32, kind="ExternalOutput").ap()

    with tile.TileContext(nc) as tc:
        # ---- whole-kernel pools ----
        cpool = tc.alloc_tile_pool(name="const", bufs=1)
        pspool = tc.alloc_tile_pool(name="ps", bufs=8, space="PSUM")
        spool = tc.alloc_tile_pool(name="small", bufs=8)

        epsT = cpool.tile([P, 1], F32, tag="eps")
        nc.vector.memset(epsT, EPS)
        # identity for PE transposes + the FFN2-tail reduction matmuls
        idb = cpool.tile([P, P], BF, tag="idb")
        make_identity(nc, idb)

        # tokens: host pre-arranged [P, N_CH]; contiguous 28B per partition
        tok = cpool.tile([P, N_CH], mybir.dt.int32, tag="tok")
        nc.sync.dma_start(out=tok, in_=tokens)

        # peT (x SX): needed for the xT8 adds right after the gathers
        peT_s = cpool.tile([P, DC, S], BF, tag="peT")
        nc.sync.dma_start(out=peT_s, in_=peT.rearrange("p (c n) -> p c n", c=DC))

        # broadcast tiles for free-axis biases / affines (rarely used)
        def load_bcast(ap_, name, dt=F32, width=D):
            t = cpool.tile([P, width], dt, tag=name)
            nc.sync.dma_start(out=t, in_=_bcast(ap_))
            return t
        bvb = load_bcast(bv, "bvb") if use_bv else None
        bob = load_bcast(bo, "bob") if use_bo else None
        b2b = load_bcast(b2, "b2b") if use_b2 else None
        g1b = load_bcast(g1, "g1b") if use_a1 else None
        bt1b = load_bcast(bt1, "bt1b") if use_a1 else None
        g2b = load_bcast(g2, "g2b") if use_a2 else None
        bt2b = load_bcast(bt2, "bt2b") if use_a2 else None
        bq_s = bk_s = None
        if use_bq:
            bq_s = cpool.tile([P, DC], F32, tag="bq_s")
            nc.sync.dma_start(out=bq_s, in_=bq.rearrange("(c p) -> p c", p=P))
        if use_bk:
            bk_s = cpool.tile([P, DC], F32, tag="bk_s")
            nc.sync.dma_start(out=bk_s, in_=bk.rearrange("(c p) -> p c", p=P))

        # ---- persistent activations ----
        # (alloc order is release order reversed: wpool/x8pool release
        # after the qk second tile; bpool+xgpool after the wo chunks)
        xgpool = tc.alloc_tile_pool(name="xg", bufs=1)
        xg = xgpool.tile([P, N_CH, D], BF, tag="xg")       # gathered emb (x SX)

        bpool = tc.alloc_tile_pool(name="attn_acts", bufs=1)
        qT = bpool.tile([P, DC, N], BF, tag="qT")
        kT = bpool.tile([P, DC, N], BF, tag="kT")
        v_aug = bpool.tile([P, BL, H * VG], BF, tag="v_aug")
        v_r = v_aug.rearrange("p b (h e) -> p b h e", e=VG)
        expT = bpool.tile([P, H, N], BF, tag="expT")
        ctx_nat = bpool.tile([P, BL, D], BF, tag="ctx_nat")
        if dbg:
            nc.vector.memset(v_aug, 0.0)   # pad regions, for debug dumps only
            nc.vector.memset(expT, 0.0)
            nc.vector.memset(ctx_nat, 0.0)

        x8pool = tc.alloc_tile_pool(name="x8p", bufs=1)
        xT8 = x8pool.tile([P, DC, N], F8, tag="xT8")       # fp8 xT (x SX)

        # wv first (earliest matmul consumer), in two ct2-contiguous
        # halves so v_batch(0) starts on half 0
        wpool = tc.alloc_tile_pool(name="wqkv", bufs=3)
        wv_s = wpool.tile([P, 2, DC, 512], F8, tag="wqkv")
        wv_r = wv.rearrange("p (t c n) -> p t c n", t=2, c=DC)
        nc.sync.dma_start(out=wv_s[:, 0], in_=wv_r[:, 0])
        nc.sync.dma_start(out=wv_s[:, 1], in_=wv_r[:, 1])

        # ---- P0: gathers (gpsimd queue) -> PE transposes -> xT8 adds ----
        def gather_issue(c):
            n0, rows = CHUNKS[c]
            nc.gpsimd.indirect_dma_start(
                out=xg[:rows, c, :], out_offset=None, in_=emb,
                in_offset=IndirectOffsetOnAxis(ap=tok[:rows, c:c + 1], axis=0))

        # wq/wk triggered from the gpsimd queue BETWEEN the gather issues:
        # their descriptors then enter the shared DMA rings after the
        # early gathers', so gather c0 (which gates all PE work) isn't
        # starved behind 2MB of weight descriptors in the per-engine FIFOs
        wq_s = wpool.tile([P, DC, D], F8, tag="wqkv")
        wk_s = wpool.tile([P, DC, D], F8, tag="wqkv")
        for c in range(N_CH):
            gather_issue(c)
        # tile_wait_until keeps the scheduler from hoisting these ahead of
        # the gathers in the pool queue (they have no data deps of their
        # own); the modeled delay only affects queue placement
        with tc.tile_wait_until(0.003):
            nc.gpsimd.dma_start(out=wq_s, in_=wq.rearrange("p (c n) -> p c n", c=DC))
        with tc.tile_wait_until(0.005):
            nc.gpsimd.dma_start(out=wk_s, in_=wk.rearrange("p (c n) -> p c n", c=DC))

        # SW-scaled identity for the Wo-phase emb-residual matmuls
        # (built on gpsimd behind the gather issues; needed only ~90us in)
        idsw = cpool.tile([P, P], BF, tag="idsw")
        nc.vector.memset(idsw, 0.0)
        nc.gpsimd.affine_select(
            out=idsw, in_=idsw, compare_op=OP.not_equal, fill=SW,
            base=0, pattern=[[-1, P]], channel_multiplier=1)

        # pe1/perm for the Wo-phase pe-residual matmuls; issued after
        # wq/wk so they don't contend with the preamble critical path
        pe1_s = cpool.tile([P, D], BF, tag="pe1")
        nc.sync.dma_start(out=pe1_s[:S], in_=pe1)
        perm_s = cpool.tile([P, N_CH, P], BF, tag="perm")
        nc.sync.dma_start(out=perm_s[:S],
                          in_=perm.rearrange("p (c r) -> p c r", c=N_CH))

        def xpose_chunk(c):
            """PE-transpose xg chunk c, then xT8 = fp8(x*SX + pe*SX)."""
            n0, rows = CHUNKS[c]
            pst = pspool.tile([P, DC, P], BF, tag="ps")
            for d in range(DC):
                nc.tensor.transpose(out=pst[:, d, 0:rows],
                                    in_=xg[:rows, c, d * P:(d + 1) * P],
                                    identity=idb[:rows, :rows])
            segs, off = [], 0
            while off < rows:
                s = (n0 + off) % S
                ln = min(rows - off, S - s)
                segs.append((off, s, ln))
                off += ln
            for (off, s, ln) in segs:
                nc.vector.tensor_add(
                    out=xT8[:, :, n0 + off:n0 + off + ln],
                    in0=pst[:, :, off:off + ln],
                    in1=peT_s[:, :, s:s + ln])

        # ---- QKV in fp8 DoubleRow ----
        def qk8(w_s, dst, b_s, on_scalar, t0, tw):
            for ct in range(DC):
                ps = pspool.tile([P, 512], F32, tag="ps")
                for k2 in range(DC // 2):
                    nc.tensor.matmul(ps[:, :tw],
                                     lhsT=w_s[:, 2 * k2:2 * k2 + 2,
                                              ct * P:(ct + 1) * P],
                                     rhs=xT8[:, 2 * k2:2 * k2 + 2, t0:t0 + tw],
                                     start=(k2 == 0), stop=(k2 == DC // 2 - 1),
                                     perf_mode=DR)
                if b_s is not None:
                    nc.scalar.activation(out=dst[:, ct, t0:t0 + tw],
                                         in_=ps[:, :tw], func=AF.Copy,
                                         bias=b_s[:, ct:ct + 1], scale=QSCALE)
                elif on_scalar:
                    nc.scalar.activation(out=dst[:, ct, t0:t0 + tw],
                                         in_=ps[:, :tw], func=AF.Copy,
                                         scale=QSCALE)
                else:
                    nc.vector.tensor_scalar_mul(out=dst[:, ct, t0:t0 + tw],
                                                in0=ps[:, :tw], scalar1=QSCALE)

        def v_batch(b):
            for ct2 in range(2):
                ps = pspool.tile([P, 512], F32, tag="ps")
                for k2 in range(DC // 2):
                    nc.tensor.matmul(ps[:S, :],
                                     lhsT=xT8[:, 2 * k2:2 * k2 + 2,
                                              b * S:(b + 1) * S],
                                     rhs=wv_s[:, ct2, 2 * k2:2 * k2 + 2, :],
                                     start=(k2 == 0), stop=(k2 == DC // 2 - 1),
                                     perf_mode=DR)
                dstv = v_r[:S, b, ct2 * 8:(ct2 + 1) * 8, 0:64]
                psv = ps[:S, :].rearrange("p (h e) -> p h e", e=64)
                if use_bv:
                    sc = spool.tile([P, 512], F32, tag="vsc", bufs=2)
                    nc.vector.tensor_scalar_mul(out=sc[:S], in0=ps[:S, :],
                                                scalar1=QSCALE)
                    nc.vector.tensor_add(
                        out=dstv, in0=sc[:S].rearrange("p (h e) -> p h e", e=64),
                        in1=bvb[:S, ct2 * 512:(ct2 + 1) * 512]
                            .rearrange("p (h e) -> p h e", e=64))
                else:
                    nc.vector.tensor_scalar_mul(out=dstv, in0=psv,
                                                scalar1=QSCALE)
            # 1/SX in the sums column: the reciprocal then yields SX/sum,
            # so ctx_nat carries a factor SX, sized for the fp8 ctxT cast
            nc.vector.memset(v_r[:S, b, :, 64:65], 1.0 / SX)

        def scores_group(hq, bq4):
            # heads 4hq..4hq+3 as two even/odd pairs; even head sits at
            # partition 0, odd at 64 -> distinct PE row groups, MMs overlap
            for pr in range(2):
                h0, h1 = hq * 4 + 2 * pr, hq * 4 + 2 * pr + 1
                pch = h0 // 2
                psA = pspool.tile([P, 4, S], F32, tag="ps")
                psB = pspool.tile([P, 4, S], F32, tag="ps")
                for j in range(4):
                    b = bq4 * 4 + j
                    sl = slice(b * S, (b + 1) * S)
                    nc.tensor.matmul(psA[:S, j, :], lhsT=kT[0:64, pch, sl],
                                     rhs=qT[0:64, pch, sl],
                                     start=True, stop=True)
                    nc.tensor.matmul(psB[:S, j, :], lhsT=kT[64:128, pch, sl],
                                     rhs=qT[64:128, pch, sl],
                                     start=True, stop=True)
                for h, psx in ((h0, psA), (h1, psB)):
                    nc.scalar.activation(
                        out=expT[:S, h, bq4 * 4 * S:(bq4 * 4 + 4) * S]
                            .rearrange("p (j s) -> p j s", s=S),
                        in_=psx[:S], func=AF.Exp, scale=float(1.0 / np.sqrt(E)))

        def ctx_b(b):
            # all 16 heads of batch b, then its ctxT transpose on the
            # (midgame-idle) DMA XBAR via the sync queue
            for hq in range(4):
                ps = pspool.tile([P, 4, VG], F32, tag="ps")
                for j in range(4):
                    h = hq * 4 + j
                    nc.tensor.matmul(ps[:S, j, 0:65],
                                     lhsT=expT[:S, h, b * S:(b + 1) * S],
                                     rhs=v_r[:S, b, h, 0:65],
                                     start=True, stop=True)
                rc = spool.tile([P, 4], F32, tag="rc")
                nc.vector.reciprocal(out=rc[:S], in_=ps[:S, :, 64])
                # one op for all 4 heads: rc broadcast along e via a
                # zero-stride free dim
                rcs = rc[:S, 0:4]
                rcb = bass.AP(tensor=rcs.tensor, offset=rcs.offset,
                              ap=list(rcs.ap) + [[0, 64]])
                nc.vector.tensor_mul(
                    out=ctx_nat[:S, b, hq * 256:(hq + 1) * 256]
                        .rearrange("p (j e) -> p j e", e=64),
                    in0=ps[:S, :, 0:64], in1=rcb)
            # ctxT via PE transposes; the fp8 cast evacuation runs on the
            # scalar engine (the midgame is DVE-bound)
            for dq in range(2):
                pst = pspool.tile([P, 4, S], BF, tag="ps")
                for j in range(4):
                    d = dq * 4 + j
                    nc.tensor.transpose(out=pst[:, j, :],
                                        in_=ctx_nat[:S, b, d * P:(d + 1) * P],
                                        identity=idb[:S, :S])
                nc.scalar.activation(
                    out=ctxT[:, dq * 4:(dq + 1) * 4, b * S:(b + 1) * S],
                    in_=pst, func=AF.Copy)

        # PE order: transposes c0-c3 interleave with v batches (each v
        # batch needs only the chunks covering its tokens); the qk tiles
        # then cover chunks 0-3 while c4-c6 gathers land
        xpose_chunk(0)
        v_batch(0)
        xpose_chunk(1)
        v_batch(1)
        xpose_chunk(2)
        v_batch(2)
        xpose_chunk(3)
        v_batch(3)
        qk8(wq_s, qT, bq_s, True, 0, 512)
        qk8(wk_s, kT, bk_s, False, 0, 512)
        xpose_chunk(4)
        xpose_chunk(5)
        xpose_chunk(6)

        # mid-phase allocations (right stack, LIFO: ctpool released first,
        # then wopool, mpool; x1fpool survives through FFN2)
        x1fpool = tc.alloc_tile_pool(name="x1f", bufs=1, side="right")
        x1b = x1fpool.tile([P, N_CH, D], BF, tag="x1b")
        x1T = x1fpool.tile([P, DC, N], BF, tag="x1T")
        mpool = tc.alloc_tile_pool(name="mid", bufs=1, side="right")
        ctxT = mpool.tile([P, DC, N], F8, tag="ctxT")
        # z in bf16: halves the LN1 DVE read traffic; the bf16 rounding
        # is well inside the error budget
        z = mpool.tile([P, N_CH, D], BF, tag="z")
        wopool = tc.alloc_tile_pool(name="wop", bufs=1, side="right")
        wo_s = wopool.tile([P, DC, D], F8, tag="wo")
        nc.sync.dma_start(out=wo_s, in_=wo.rearrange("p (c n) -> p c n", c=DC))

        for hq in range(4):
            scores_group(hq, 0)
        # ctx batches 0-3 and v4 depend only on bq0 scores — they fill
        # the PE while the second qk tile waits on chunks 4-6
        ctx_b(0)
        ctx_b(1)
        ctx_b(2)
        ctx_b(3)
        v_batch(4)
        v_batch(5)
        v_batch(6)
        qk8(wq_s, qT, bq_s, True, 512, N - 512)
        qk8(wk_s, kT, bk_s, False, 512, N - 512)

        def wo_chunk(c):
            n0, rows = CHUNKS[c]
            for ct2 in range(2):
                ps = pspool.tile([P, 512], F32, tag="ps")
                for k2 in range(DC // 2):
                    nc.tensor.matmul(ps[:rows],
                                     lhsT=ctxT[:, 2 * k2:2 * k2 + 2,
                                               n0:n0 + rows],
                                     rhs=wo_s[:, 2 * k2:2 * k2 + 2,
                                              ct2 * 512:(ct2 + 1) * 512],
                                     start=(k2 == 0), stop=False,
                                     perf_mode=DR)
                # residual: += SX*SW*emb via the SW-scaled identity over
                # xg (which carries SX), then += SX*SW*pe via the chunk's
                # row-permutation against the single [100, D] pe tile
                nc.tensor.matmul(ps[:rows], lhsT=idsw[:rows, :rows],
                                 rhs=xg[:rows, c, ct2 * 512:(ct2 + 1) * 512],
                                 start=False, stop=False)
                nc.tensor.matmul(ps[:rows], lhsT=perm_s[:S, c, 0:rows],
                                 rhs=pe1_s[:S, ct2 * 512:(ct2 + 1) * 512],
                                 start=False, stop=True)
                nc.scalar.activation(out=z[:rows, c, ct2 * 512:(ct2 + 1) * 512],
                                     in_=ps[:rows], func=AF.Copy, scale=QSCALE)
            if use_bo:
                nc.vector.tensor_add(out=z[:rows, c, :], in0=z[:rows, c, :],
                                     in1=bob[:rows])

        def resid_mm(ps, rows, resid):
            # += residual via identity matmul (diag-hit contraction over
            # the token partitions); closes the accumulation group
            nc.tensor.matmul(ps[:rows], lhsT=idb[:rows, :rows], rhs=resid,
                             start=False, stop=True)

        # LN1 stats helpers (needed early: the stats thread into the wo
        # chain below; the normalize half lives after the x1 pools)
        mvs = []
        def ln1_stats(c):
            rows = CHUNKS[c][1]
            st = spool.tile([P, 2, 6], F32, tag="st")
            mv = spool.tile([P, 2], F32, tag="mv")
            src = z[:rows, c, :]
            nc.vector.bn_stats(out=st[:rows, 0, :], in_=src[:, 0:512])
            nc.vector.bn_stats(out=st[:rows, 1, :], in_=src[:, 512:1024])
            nc.vector.bn_aggr(out=mv[:rows], in_=st[:rows])
            mvs.append(mv)

        def ln1_sqrt(c):
            rows = CHUNKS[c][1]
            nc.scalar.activation(out=mvs[c][:rows, 1:2], in_=mvs[c][:rows, 1:2],
                                 func=AF.Sqrt, bias=epsT[:rows], scale=1.0)

        def ln1_norm(c):
            n0, rows = CHUNKS[c]
            mv = mvs[c]
            nc.vector.reciprocal(out=mv[:rows, 1:2], in_=mv[:rows, 1:2])
            nc.vector.tensor_scalar(out=x1b[:rows, c, :], in0=z[:rows, c, :],
                                    scalar1=mv[:rows, 0:1],
                                    scalar2=mv[:rows, 1:2],
                                    op0=OP.subtract, op1=OP.mult)
            if use_a1:
                nc.vector.tensor_mul(out=x1b[:rows, c, :],
                                     in0=x1b[:rows, c, :], in1=g1b[:rows])
                nc.vector.tensor_add(out=x1b[:rows, c, :],
                                     in0=x1b[:rows, c, :], in1=bt1b[:rows])
            for dq in range(2):
                pst = pspool.tile([P, 512], BF, tag="ps")
                for j in range(4):
                    d = dq * 4 + j
                    nc.tensor.transpose(out=pst[:, j * rows:(j + 1) * rows],
                                        in_=x1b[:rows, c, d * P:(d + 1) * P],
                                        identity=idb[:rows, :rows])
                nc.vector.tensor_copy(
                    out=x1T[:, dq * 4:(dq + 1) * 4, n0:n0 + rows],
                    in_=pst[:, 0:4 * rows].rearrange("p (j r) -> p j r",
                                                     r=rows))

        # interleave: fp8 v batches and ACT-paced scores groups spread
        # against the vector-paced ctx evacuations; wo chunks start as soon
        # as their two ctxT batches exist; the LN1 chain (stats -> sqrt ->
        # norm -> XBAR transpose) threads in behind them so x1T chunks are
        # ready the moment FFN1 starts
        scores_group(0, 1)
        wo_chunk(0)
        scores_group(1, 1)
        wo_chunk(1)
        v_batch(7)
        scores_group(2, 1)
        wo_chunk(2)
        scores_group(3, 1)
        ln1_stats(0)
        ln1_sqrt(0)
        ctx_b(4)
        ln1_stats(1)
        ln1_sqrt(1)
        ln1_norm(0)
        ctx_b(5)
        ln1_stats(2)
        ln1_sqrt(2)
        wo_chunk(3)
        ln1_norm(1)
        ctx_b(6)
        wo_chunk(4)
        ln1_stats(3)
        ln1_sqrt(3)
        ln1_norm(2)
        ctx_b(7)
        wo_chunk(5)
        ln1_norm(3)
        wo_chunk(6)
        ln1_stats(4)
        ln1_sqrt(4)
        ln1_norm(4)
        ln1_stats(5)
        ln1_sqrt(5)
        ln1_norm(5)
        ln1_stats(6)
        ln1_sqrt(6)
        ln1_norm(6)
        if dbg:
            d_xT8 = nc.dram_tensor("d_xT8", [P, DC * N], F8, kind="ExternalOutput").ap()
            d_qT = nc.dram_tensor("d_qT", [P, DC * N], BF, kind="ExternalOutput").ap()
            d_kT = nc.dram_tensor("d_kT", [P, DC * N], BF, kind="ExternalOutput").ap()
            d_v = nc.dram_tensor("d_v", [P, BL * H * VG], BF, kind="ExternalOutput").ap()
            d_exp = nc.dram_tensor("d_exp", [P, H * N], BF, kind="ExternalOutput").ap()
            d_ctx = nc.dram_tensor("d_ctx", [P, BL * D], BF, kind="ExternalOutput").ap()
            d_ctxT = nc.dram_tensor("d_ctxT", [P, DC * N], F8, kind="ExternalOutput").ap()
            d_z = nc.dram_tensor("d_z", [P, N_CH * D], BF, kind="ExternalOutput").ap()
            nc.sync.dma_start(out=d_xT8, in_=xT8.rearrange("p c n -> p (c n)"))
            nc.sync.dma_start(out=d_qT, in_=qT.rearrange("p c n -> p (c n)"))
            nc.sync.dma_start(out=d_kT, in_=kT.rearrange("p c n -> p (c n)"))
            nc.sync.dma_start(out=d_v, in_=v_aug.rearrange("p b h -> p (b h)"))
            nc.sync.dma_start(out=d_exp, in_=expT.rearrange("p h n -> p (h n)"))
            nc.sync.dma_start(out=d_ctx, in_=ctx_nat.rearrange("p b d -> p (b d)"))
            nc.sync.dma_start(out=d_ctxT, in_=ctxT.rearrange("p c n -> p (c n)"))
            nc.sync.dma_start(out=d_z, in_=z.rearrange("p c d -> p (c d)"))
        wopool.release()
        mpool.release()
        wpool.release()
        x8pool.release()
        bpool.release()
        xgpool.release()

        # ---- FFN1: h1T = relu(W1.T @ x1T + b1)  (bf16, T layout) ----
        w2pool = tc.alloc_tile_pool(name="w2p", bufs=1)
        w2_s = w2pool.tile([P, FC, D], BF, tag="w2s")
        hpool = tc.alloc_tile_pool(name="h1", bufs=1, side="right")
        h1T = hpool.tile([P, FC, N], BF, tag="h1T")
        b1_s = None
        if use_b1:
            b1_s = cpool.tile([P, FC], F32, tag="b1_s")
            nc.sync.dma_start(out=b1_s, in_=b1.rearrange("(c p) -> p c", p=P))
        w1_r = w1.rearrange("p (c f) -> p c f", c=DC)
        N_TILES = [(0, 512), (512, N - 512)]
        with tc.tile_pool(name="w1s", bufs=4) as w1pool:
            def f1_mm(w1t, fg, fc4, t0, tw):
                fabs = fg * 4 + fc4
                ps = pspool.tile([P, 512], F32, tag="ps")
                for kc in range(DC):
                    nc.tensor.matmul(ps[:, :tw],
                                     lhsT=w1t[:, kc, fc4 * P:(fc4 + 1) * P],
                                     rhs=x1T[:, kc, t0:t0 + tw],
                                     start=(kc == 0), stop=(kc == DC - 1))
                if use_b1:
                    nc.scalar.activation(out=h1T[:, fabs, t0:t0 + tw],
                                         in_=ps[:, :tw], func=AF.Relu,
                                         bias=b1_s[:, fabs:fabs + 1], scale=1.0)
                else:
                    nc.scalar.activation(out=h1T[:, fabs, t0:t0 + tw],
                                         in_=ps[:, :tw], func=AF.Relu)

            # two passes: all f-groups on n-tile 0 (tokens 0-512) first, so
            # the trailing LN1 chunks 4-6 and their transposes hide under
            # ~50us of pass-1 matmuls; w1 is streamed twice (DMA is cheap)
            w2_r = w2.rearrange("p (c n) -> p c n", c=FC)
            for pi, (t0, tw) in enumerate(N_TILES):
                for fg in range(8):
                    # w2 halves ride in pass-1's DMA slack (pass-2 has none)
                    if pi == 0 and fg == 3:
                        nc.sync.dma_start(out=w2_s[:, 0:16, :],
                                          in_=w2_r[:, 0:16, :])
                    elif pi == 0 and fg == 6:
                        nc.sync.dma_start(out=w2_s[:, 16:32, :],
                                          in_=w2_r[:, 16:32, :])
                    w1t = w1pool.tile([P, DC, 512], BF, tag="w1t")
                    nc.sync.dma_start(out=w1t,
                                      in_=w1_r[:, :, fg * 512:(fg + 1) * 512])
                    if pi == 0 and fg == 0:
                        # first f-group in two 256-token sub-tiles: the
                        # first needs only x1T chunks 0-1, so FFN1 starts
                        # a couple of LN1-norms earlier
                        for fc4 in range(4):
                            f1_mm(w1t, fg, fc4, 0, 256)
                        for fc4 in range(4):
                            f1_mm(w1t, fg, fc4, 256, 256)
                    else:
                        for fc4 in range(4):
                            f1_mm(w1t, fg, fc4, t0, tw)

        # ---- FFN2 + residual + LN2 -> out ----
        # tail chunk (32 rows) first: its packed-reduction latency hides
        # under the full chunks instead of extending the kernel tail.
        # The final chunk's LN2 runs straight off PSUM (no z2t copy).
        opool = tc.alloc_tile_pool(name="ostage", bufs=3)
        order = [N_CH - 1] + list(range(N_CH - 1))
        for oi, c in enumerate(order):
            last = oi == len(order) - 1
            n0, rows = CHUNKS[c]
            z2t = None if last else opool.tile([P, D], F32, tag="z2")
            st = spool.tile([P, 2, 6], F32, tag="st")
            pss = []
            for ct2 in range(2):
                ps = pspool.tile([P, 512], F32, tag="ps")
                if rows == P:
                    for kc in range(FC):
                        nc.tensor.matmul(ps[:rows],
                                         lhsT=h1T[:, kc, n0:n0 + rows],
                                         rhs=w2_s[:, kc,
                                                  ct2 * 512:(ct2 + 1) * 512],
                                         start=(kc == 0), stop=False)
                    resid_mm(ps, rows,
                             x1b[:rows, c, ct2 * 512:(ct2 + 1) * 512])
                    if last:
                        nc.vector.bn_stats(out=st[:rows, ct2, :], in_=ps[:rows])
                        pss.append(ps)
                    else:
                        nc.vector.tensor_copy(
                            out=z2t[:rows, ct2 * 512:(ct2 + 1) * 512],
                            in_=ps[:rows])
                        nc.vector.bn_stats(
                            out=st[:rows, ct2, :],
                            in_=z2t[:rows, ct2 * 512:(ct2 + 1) * 512])
                else:
                    # 32-row tail: 4 col-groups accumulate 8-kc partial sums
                    # concurrently (kk outer interleaves the chains); the
                    # partition groups are then summed with 4 accumulating
                    # identity-slice matmuls (walrus forbids cross-partition
                    # DVE operands, the PE reduction sidesteps that)
                    for kk in range(8):
                        for g in range(4):
                            kc = g * 8 + kk
                            nc.tensor.matmul(
                                ps[g * 32:g * 32 + 32, :],
                                lhsT=h1T[:, kc, n0:n0 + rows],
                                rhs=w2_s[:, kc, ct2 * 512:(ct2 + 1) * 512],
                                start=(kk == 0), stop=(kk == 7),
                                tile_position=(0, g * 32),
                                skip_group_check=True)
                    tsb = spool.tile([P, 512], BF, tag="acc", bufs=2)
                    nc.vector.tensor_copy(out=tsb, in_=ps)
                    ps2 = pspool.tile([P, 512], F32, tag="ps")
                    for g in range(4):
                        nc.tensor.matmul(ps2[0:32, :],
                                         lhsT=idb[:, g * 32:g * 32 + 32],
                                         rhs=tsb,
                                         start=(g == 0), stop=False)
                    resid_mm(ps2, rows,
                             x1b[:rows, c, ct2 * 512:(ct2 + 1) * 512])
                    nc.vector.tensor_copy(
                        out=z2t[:rows, ct2 * 512:(ct2 + 1) * 512],
                        in_=ps2[:rows])
                    nc.vector.bn_stats(
                        out=st[:rows, ct2, :],
                        in_=z2t[:rows, ct2 * 512:(ct2 + 1) * 512])
            if use_b2:
                nc.vector.tensor_add(out=z2t[:rows], in0=z2t[:rows], in1=b2b[:rows])
                nc.vector.bn_stats(out=st[:rows, 0, :], in_=z2t[:rows, 0:512])
                nc.vector.bn_stats(out=st[:rows, 1, :], in_=z2t[:rows, 512:1024])
            ot = opool.tile([P, D], F32, tag="ot")
            mv = spool.tile([P, 2], F32, tag="mv")
            nc.vector.bn_aggr(out=mv[:rows], in_=st[:rows])
            nc.scalar.activation(out=mv[:rows, 1:2], in_=mv[:rows, 1:2],
                                 func=AF.Sqrt, bias=epsT[:rows], scale=1.0)
            nc.vector.reciprocal(out=mv[:rows, 1:2], in_=mv[:rows, 1:2])
            # normalize + store per half so the final DMA overlaps the
            # second half's normalize
            for h2 in range(2):
                sl = slice(h2 * 512, (h2 + 1) * 512)
                src = pss[h2][:rows] if last else z2t[:rows, sl]
                nc.vector.tensor_scalar(out=ot[:rows, sl], in0=src,
                                        scalar1=mv[:rows, 0:1],
                                        scalar2=mv[:rows, 1:2],
                                        op0=OP.subtract, op1=OP.mult)
                if use_a2:
                    nc.vector.tensor_mul(out=ot[:rows, sl], in0=ot[:rows, sl],
                                         in1=g2b[:rows, sl])
                    nc.vector.tensor_add(out=ot[:rows, sl], in0=ot[:rows, sl],
                                         in1=bt2b[:rows, sl])
                nc.sync.dma_start(out=out[n0:n0 + rows, sl],
                                  in_=ot[:rows, sl])

        opool.release()
        w2pool.release()
        hpool.release()
        x1fpool.release()
        spool.release()
        pspool.release()
        cpool.release()

    nc.compile()
    return nc


# ---------------- host side ----------------

def _positional_encoding(seq_len, dim):
    pos = np.arange(seq_len).reshape(seq_len, 1).astype(np.float64)
    i = np.arange(dim)
    div_term = np.power(10000.0, 2 * (i // 2) / dim)
    pe = np.zeros((seq_len, dim))
    pe[:, 0::2] = np.sin(pos / div_term[0::2])
    pe[:, 1::2] = np.cos(pos / div_term[1::2])
    return pe.astype(np.float32)


_NC_CACHE = {}


def _get_nc(flags):
    key = tuple(sorted(flags.items()))
    if key not in _NC_CACHE:
        _NC_CACHE[key] = build_nc(flags)
    return _NC_CACHE[key]


def make_in_maps(tokens, emb_table, Wq, bq, Wk, bk, Wv, bv, Wo, bo,
                 W1, b1, W2, b2, gamma1, beta1, gamma2, beta2):
    bf16 = ml_dtypes.bfloat16
    fp8 = ml_dtypes.float8_e4m3
    f32 = np.float32

    def arrange(w, nchunk):  # [rows, n] -> [P, nchunk*n] in SBUF layout
        rows, n = w.shape
        return np.ascontiguousarray(
            w.reshape(nchunk, P, n).swapaxes(0, 1).reshape(P, nchunk * n))

    def merge_hw(w):  # [H, D, E] -> [D, H*E]
        return np.transpose(np.asarray(w, f32), (1, 0, 2)).reshape(D, D)

    def to_fp8(w):  # scale, clip below e4m3 max-finite, quantize
        return np.clip(w * SW, -240.0, 240.0).astype(fp8)

    flags = {
        "bq": bool(np.any(np.asarray(bq))), "bk": bool(np.any(np.asarray(bk))),
        "bv": bool(np.any(np.asarray(bv))), "bo": bool(np.any(np.asarray(bo))),
        "b1": bool(np.any(np.asarray(b1))), "b2": bool(np.any(np.asarray(b2))),
        "a1": not (np.all(np.asarray(gamma1) == 1.0) and not np.any(np.asarray(beta1))),
        "a2": not (np.all(np.asarray(gamma2) == 1.0) and not np.any(np.asarray(beta2))),
    }

    pe1 = _positional_encoding(S, D)
    # wv in two ct2-contiguous halves (the kernel loads them separately)
    wv_a = arrange(to_fp8(merge_hw(Wv)), DC)                    # [P, DC*D]
    wv_a = wv_a.reshape(P, DC, 2, 512).transpose(0, 2, 1, 3)    # [P, 2, DC, 512]
    # chunk-row permutation matrices: perm[s, c*128+r] = 1 iff the pe row
    # for token c*128+r is s (pe repeats every S tokens)
    perm = np.zeros((S, N_CH * P), f32)
    for c in range(N_CH):
        n0, rows = c * P, min(P, N - c * P)
        r = np.arange(rows)
        perm[(n0 + r) % S, c * P + r] = 1.0
    common = {
        "emb": (np.asarray(emb_table, f32) * SX).astype(bf16),
        "pe1": (pe1 * np.float32(SX * SW)).astype(bf16),
        "perm": perm.astype(bf16),
        "peT": arrange(np.ascontiguousarray(pe1.T * SX).astype(bf16), DC),
        "wq": arrange(to_fp8(merge_hw(Wq)), DC),
        "wk": arrange(to_fp8(merge_hw(Wk)), DC),
        "wv": np.ascontiguousarray(wv_a.reshape(P, 2 * DC * 512)),
        "wo": arrange(to_fp8(np.asarray(Wo, f32)), DC),
        "w1": arrange(np.asarray(W1, f32).astype(bf16), DC),
        "w2": arrange(np.asarray(W2, f32).astype(bf16), FC),
    }
    if flags["bq"]: common["bq"] = np.asarray(bq, f32).reshape(D)
    if flags["bk"]: common["bk"] = np.asarray(bk, f32).reshape(D)
    if flags["bv"]: common["bv"] = np.asarray(bv, f32).reshape(D)
    if flags["bo"]: common["bo"] = np.asarray(bo, f32).reshape(D)
    if flags["b1"]: common["b1"] = np.asarray(b1, f32).reshape(F)
    if flags["b2"]: common["b2"] = np.asarray(b2, f32).reshape(D)
    if flags["a1"]:
        common["g1"] = np.asarray(gamma1, f32).reshape(D)
        common["bt1"] = np.asarray(beta1, f32).reshape(D)
    if flags["a2"]:
        common["g2"] = np.asarray(gamma2, f32).reshape(D)
        common["bt2"] = np.asarray(beta2, f32).reshape(D)

    tokens = np.asarray(tokens, np.int32)
    in_maps = []
    for i in range(NCORES):
        flat = tokens[i * BL:(i + 1) * BL].reshape(N)
        padded = np.zeros(N_CH * P, np.int32)
        padded[:N] = flat
        m = dict(common)
        # tok[p, c] = token index c*128+p
        m["tokens"] = np.ascontiguousarray(padded.reshape(N_CH, P).T)
        in_maps.append(m)
    return flags, in_maps


def kernel(**inputs):
    flags, in_maps = make_in_maps(**inputs)
    nc = _get_nc(flags)
    res = run_bass_kernel_spmd(nc, in_maps, list(range(NCORES)))
    outs = [np.asarray(res.results[i]["out"], np.float32).reshape(BL, S, D)
            for i in range(NCORES)]
    return np.concatenate(outs, axis=0)
